# revision 10
# baseline (speedup 1.0000x reference)
"""Trainium2 Bass kernel for nn_MultiHeadGATMethod (4-head GAT message passing).

v2: f16 tables, 4 per-chunk KH tables, CALL=2048 gathers, host-built one-hot
S tiles (fp8 content in bf16-typed storage), batched per-call DVE ops with an
f16 tree score reduction, exp straight into the rhs z-columns, and a
DVE/ACT-balanced drain.
"""
import sys
sys.path.insert(0, "/opt/trn_rl_repo")
sys.path.insert(0, "/root/.axon_site/_ro/trn_rl_repo")

import time
import numpy as np
import jax
from jax.sharding import Mesh, PartitionSpec, NamedSharding
from jax.experimental.shard_map import shard_map

import concourse.mybir as mybir
from concourse import bass2jax
from concourse.bass2jax import _bass_exec_p, partition_id_tensor, install_neuronx_cc_hook


class AxRunner:
    def __init__(self, nc, n_cores):
        install_neuronx_cc_hook()
        self.nc = nc
        self.n_cores = n_cores
        partition_name = nc.partition_id_tensor.name if nc.partition_id_tensor else None
        in_names, out_names, out_avals, zero_outs = [], [], [], []
        for alloc in nc.m.functions[0].allocations:
            if not isinstance(alloc, mybir.MemoryLocationSet):
                continue
            name = alloc.memorylocations[0].name
            if alloc.kind == "ExternalInput":
                if name != partition_name:
                    in_names.append(name)
            elif alloc.kind == "ExternalOutput":
                out_names.append(name)
                shape = tuple(alloc.tensor_shape)
                dtype = mybir.dt.np(alloc.dtype)
                out_avals.append(jax.core.ShapedArray(shape, dtype))
                zero_outs.append((shape, dtype))
        self.in_names, self.out_names = in_names, out_names
        self.out_avals = out_avals
        n_params = len(in_names)
        all_in_names = list(in_names) + list(out_names)
        if partition_name is not None:
            all_in_names.append(partition_name)

        def _body(*args):
            operands = list(args)
            if partition_name is not None:
                operands.append(partition_id_tensor())
            outs = _bass_exec_p.bind(
                *operands,
                out_avals=tuple(out_avals),
                in_names=tuple(all_in_names),
                out_names=tuple(out_names),
                lowering_input_output_aliases=(),
                sim_require_finite=False,
                sim_require_nnan=False,
                nc=nc,
            )
            return tuple(outs)

        devices = jax.devices()[:n_cores]
        self.mesh = Mesh(np.asarray(devices), ("core",))
        spec = PartitionSpec("core")
        in_specs = (spec,) * (n_params + len(out_names))
        out_specs = (spec,) * len(out_names)
        self.fn = jax.jit(
            shard_map(_body, mesh=self.mesh, in_specs=in_specs,
                      out_specs=out_specs, check_rep=False),
            keep_unused=True,
        )
        self.sharding = NamedSharding(self.mesh, spec)
        self._dev_in = None

    def put_inputs(self, in_maps):
        """in_maps: list (len n_cores) of {name: np.ndarray}."""
        concat = [
            np.concatenate([np.asarray(in_maps[c][nm]) for c in range(self.n_cores)], axis=0)
            for nm in self.in_names
        ]
        self._dev_in = [jax.device_put(a, self.sharding) for a in concat]
        self._dev_zeros = [
            jax.device_put(
                np.zeros((self.n_cores * s[0], *s[1:]), d), self.sharding
            )
            for (s, d) in [( (a[0]), a[1]) for a in [( (za[0]), za[1]) for za in []]]
        ]
        # build zero outputs (device-resident, re-usable because no donation)
        self._dev_zeros = []
        for aval_i, (shape, dtype) in enumerate(
            [(tuple(av.shape), av.dtype) for av in self.out_avals]
        ):
            z = np.zeros((self.n_cores * shape[0], *shape[1:]), dtype)
            self._dev_zeros.append(jax.device_put(z, self.sharding))
        jax.block_until_ready(self._dev_in)

    def run(self):
        outs = self.fn(*self._dev_in, *self._dev_zeros)
        jax.block_until_ready(outs)
        return outs

    def results(self, outs):
        res = []
        for c in range(self.n_cores):
            d = {}
            for i, nm in enumerate(self.out_names):
                a = np.asarray(outs[i])
                shape = tuple(self.out_avals[i].shape)
                d[nm] = a.reshape(self.n_cores, *shape)[c]
            res.append(d)
        return res

    def time_runs(self, iters=5):
        self.run()  # warmup/compile
        ts = []
        for _ in range(iters):
            t0 = time.perf_counter()
            self.run()
            ts.append(time.perf_counter() - t0)
        return min(ts), ts





import numpy as np
import ml_dtypes

import concourse.bass as bass
import concourse.bacc as bacc
import concourse.mybir as mybir
import concourse.tile as tile

BF16 = mybir.dt.bfloat16
F32 = mybir.dt.float32
F16 = mybir.dt.float16
FP8 = mybir.dt.float8e3
I16 = mybir.dt.int16

N_CORES = 8
N_HEADS = 4
OUT_DIM = 64
IN_DIM = 128
NEG_SLOPE = 0.2
NCHUNK = 4
TILE = 128
CALL = 1024
QW = N_HEADS * OUT_DIM          # 256
RHSW = QW + N_HEADS             # 260

# ---- dtype config ----
K_DT = F16     # score-side K~ table segment
H_DT = F16     # aggregation H table segment
Q_DT = F16     # Q~ table (gathered by destination)
PRELU_EMU = False  # emulate Prelu on DVE (CoreSim exec has no Prelu)

_DT_BYTES = {BF16: 2, F16: 2, FP8: 1, F32: 4}
_DT_NP = {BF16: ml_dtypes.bfloat16, F16: np.float16, FP8: ml_dtypes.float8_e3m4,
          F32: np.float32}
KB = QW * _DT_BYTES[K_DT]       # K seg bytes
HB = QW * _DT_BYTES[H_DT]       # H seg bytes
ROWB = KB + HB                  # gather row bytes
ROW2 = ROWB // 2                # row size in bf16 elements (tables typed bf16)
KE2 = KB // 2                   # K seg size in bf16 elements
SC2 = CALL // 2                 # s_tiles row in bf16 elements


# ---------------------------------------------------------------- host side

def preprocess(x, edge_index, Wq, Wk, Wh, bh, tiles_per_chunk=3):
    N = x.shape[0]
    E = edge_index.shape[1]
    row = np.asarray(edge_index[0], dtype=np.int64)
    col = np.asarray(edge_index[1], dtype=np.int64)
    perm = np.argsort(row, kind="stable")
    rs = row[perm].astype(np.int32)
    cs = col[perm].astype(np.int32)

    nb = [0]
    for c in range(1, N_CORES):
        t = (E * c) // N_CORES
        node = int(rs[t])
        node = max(node, nb[-1] + 1)
        nb.append(min(node, N - 1))
    nb.append(N)
    e0 = [int(np.searchsorted(rs, nb[c], "left")) for c in range(N_CORES)] + [E]

    CAP = tiles_per_chunk * TILE
    cores = []
    for c in range(N_CORES):
        lo, hi = e0[c], e0[c + 1]
        rl = rs[lo:hi] - nb[c]
        cl = cs[lo:hi]
        nloc = nb[c + 1] - nb[c]
        key = rl.astype(np.int64) * NCHUNK + (cl % NCHUNK)
        cnt = np.bincount(key, minlength=nloc * NCHUNK).reshape(nloc, NCHUNK)
        # first-fit-decreasing node->block packing (permutes local node ids)
        order = np.argsort(-cnt.max(axis=1), kind="stable")
        nb_max = nloc
        baccs_arr = np.zeros((nb_max, NCHUNK), dtype=np.int64)
        bsizes = np.zeros(nb_max, dtype=np.int64)
        nblk = 0
        bmembers = []
        for n_ in order:
            feas = np.flatnonzero(
                (bsizes[:nblk] < TILE)
                & np.all(baccs_arr[:nblk] + cnt[n_] <= CAP, axis=1))
            if len(feas):
                bi = int(feas[0])
                baccs_arr[bi] += cnt[n_]
                bsizes[bi] += 1
                bmembers[bi].append(n_)
            else:
                assert np.all(cnt[n_] <= CAP), "single node exceeds chunk cap"
                baccs_arr[nblk] = cnt[n_]
                bsizes[nblk] = 1
                bmembers.append([n_])
                nblk += 1
        nperm = np.concatenate([np.asarray(m, dtype=np.int64) for m in bmembers])
        inv = np.empty(nloc, dtype=np.int64)
        inv[nperm] = np.arange(nloc)
        rl = inv[rl]
        blocks = []
        s = 0
        for m in bmembers:
            blocks.append((s, len(m)))
            s += len(m)
        cores.append(dict(nb=nb[c], nloc=nloc, rl=rl, cl=cl, blocks=blocks,
                          nperm=nperm))

    B = max(len(ci["blocks"]) for ci in cores)
    NLOC = max(ci["nloc"] for ci in cores)
    NLOC = -(-NLOC // TILE) * TILE
    SC = B * CAP                     # chunk slots per chunk
    SCP = -(-SC // CALL) * CALL
    ncalls = SCP // CALL
    # per-chunk node counts (chunk = src % NCHUNK)
    NK = -(-N // NCHUNK)             # max rows per chunk table
    NKPAD = -(-NK // TILE) * TILE

    meta = dict(N=N, E=E, NKPAD=NKPAD, NLOC=NLOC, B=B, SC=SC, SCP=SCP,
                ncalls=ncalls, tiles_per_chunk=tiles_per_chunk, CAP=CAP,
                nb=nb, cores=cores)

    xb = np.asarray(x, dtype=np.float32)
    # chunk-grouped x^T: for chunk k, columns are nodes k, k+4, ... padded
    xTc = np.zeros((NCHUNK, IN_DIM, NKPAD), dtype=ml_dtypes.bfloat16)
    nrows_k = []
    for k in range(NCHUNK):
        idx = np.arange(k, N, NCHUNK)
        nrows_k.append(len(idx))
        xTc[k, :, :len(idx)] = xb[idx].T.astype(ml_dtypes.bfloat16)
    Wk_ = np.asarray(Wk, np.float32)
    Wh_ = np.asarray(Wh, np.float32)
    Wq_ = np.asarray(Wq, np.float32)
    w_kh = np.concatenate(
        [Wk_[h].T for h in range(N_HEADS)] + [Wh_[h].T for h in range(N_HEADS)],
        axis=1).astype(ml_dtypes.bfloat16)          # [128, 512]
    w_q = np.concatenate([Wq_[h].T for h in range(N_HEADS)], axis=1).astype(
        ml_dtypes.bfloat16)                          # [128, 256]

    per_core = []
    for c in range(N_CORES):
        ci = cores[c]
        nloc, rl, cl, blocks = ci["nloc"], ci["rl"], ci["cl"], ci["blocks"]
        chunk = (cl % NCHUNK).astype(np.int64)
        blk_of_node = np.zeros(nloc, dtype=np.int64)
        for bi, (s, n) in enumerate(blocks):
            blk_of_node[s:s + n] = bi
        ekey = (blk_of_node[rl] * NCHUNK + chunk)
        eperm = np.argsort(ekey, kind="stable")
        rl2 = rl[eperm]
        cl2 = cl[eperm]
        ekey2 = ekey[eperm]
        kh_idx = np.zeros((NCHUNK, SCP), dtype=np.int16)
        qx_idx = np.zeros((NCHUNK, SCP), dtype=np.int16)
        rrel = np.full((NCHUNK, SCP), -1, dtype=np.int32)   # -1 = ghost
        bounds = np.searchsorted(ekey2, np.arange(len(blocks) * NCHUNK + 1))
        for bi in range(len(blocks)):
            s_node = blocks[bi][0]
            for k in range(NCHUNK):
                a, b_ = bounds[bi * NCHUNK + k], bounds[bi * NCHUNK + k + 1]
                n_e = b_ - a
                assert n_e <= CAP
                base = bi * CAP
                kh_idx[k, base:base + n_e] = (cl2[a:b_] // NCHUNK).astype(np.int16)
                qx_idx[k, base:base + n_e] = rl2[a:b_].astype(np.int16)
                rrel[k, base:base + n_e] = (rl2[a:b_] - s_node)

        # idx arrays wrapped in 16 partitions, replicated to 128
        def wrap(a):
            w = a.reshape(NCHUNK, ncalls, CALL // 16, 16).transpose(0, 1, 3, 2)
            return np.broadcast_to(
                w[:, :, None, :, :], (NCHUNK, ncalls, 8, 16, CALL // 16)
            ).reshape(NCHUNK, ncalls, TILE, CALL // 16).copy()
        kh_in = wrap(kh_idx)
        qx_in = wrap(qx_idx)

        # host-built one-hot S tiles: [NCHUNK, ncalls, 128, CALL] fp8e3
        # S[p, tile*128 + d] = 1 iff rrel(slot p of tile) == d
        s_tiles = np.zeros((NCHUNK, ncalls, TILE, CALL), dtype=np.float16)
        rr = rrel.reshape(NCHUNK, ncalls, CALL // TILE, TILE)
        kk, jj, tt, pp = np.nonzero(rr >= 0)
        dd = rr[kk, jj, tt, pp]
        s_tiles[kk, jj, pp, tt * TILE + dd] = 1.0

        xTloc = np.zeros((IN_DIM, NLOC), dtype=ml_dtypes.bfloat16)
        sl = xb[nb[c]:nb[c + 1]][ci["nperm"]].T.astype(ml_dtypes.bfloat16)
        xTloc[:, :nloc] = sl
        per_core.append(dict(xTc=xTc, xTloc=xTloc, w_kh=w_kh, w_q=w_q,
                             kh_idx=kh_in, qx_idx=qx_in, s_tiles=s_tiles))
    return meta, per_core


def assemble(meta, results, bh):
    N = meta["N"]
    out = np.zeros((N, OUT_DIM), dtype=np.float32)
    bias = np.asarray(bh, np.float32).mean(axis=0)
    deg = np.zeros(N, dtype=np.int64)
    for c in range(N_CORES):
        ci = meta["cores"][c]
        res = np.asarray(results[c]["res"], dtype=np.float32)
        for bi, (s, n) in enumerate(ci["blocks"]):
            rows = meta["nb"][c] + ci["nperm"][s:s + n]
            out[rows] = res[bi * TILE: bi * TILE + n]
        deg_l = np.bincount(ci["rl"], minlength=ci["nloc"])
        deg[meta["nb"][c]: meta["nb"][c] + ci["nloc"]] = deg_l
    out += bias[None, :]
    out[deg == 0] = 0.0
    return out


# -------------------------------------------------------------- device side

def _prelu(nc, dp, out_ap, in_ap, tag):
    """Prelu out_ap = lrelu(in_ap); ACT normally, DVE emulation for CoreSim."""
    if not PRELU_EMU:
        nc.scalar.activation(out_ap, in_ap,
                             mybir.ActivationFunctionType.Prelu,
                             alpha=NEG_SLOPE)
    else:
        shp = list(in_ap.shape)
        t = dp.tile(shp, F32, tag="pr_emu")
        nc.vector.tensor_scalar(out=t[:], in0=in_ap, scalar1=NEG_SLOPE,
                                scalar2=None, op0=mybir.AluOpType.mult)
        nc.vector.tensor_tensor(out=out_ap, in0=in_ap, in1=t[:],
                                op=mybir.AluOpType.max)


def build_kernel(meta, n_cores=N_CORES, nq=2):
    NKPAD, NLOC, B = meta["NKPAD"], meta["NLOC"], meta["B"]
    SCP, ncalls, tpc = meta["SCP"], meta["ncalls"], meta["tiles_per_chunk"]
    CAP = tpc * TILE
    CPB = CALL // TILE               # tiles per gather call (16)
    KHW = 2 * QW                     # 512 f32 cols in dense matmul

    nc = bacc.Bacc("TRN2", target_bir_lowering=False, debug=False,
                   num_devices=n_cores, num_swdge_queues=nq)
    xTc = nc.dram_tensor("xTc", [NCHUNK, IN_DIM, NKPAD], BF16, kind="ExternalInput")
    xTloc = nc.dram_tensor("xTloc", [IN_DIM, NLOC], BF16, kind="ExternalInput")
    w_kh = nc.dram_tensor("w_kh", [IN_DIM, KHW], BF16, kind="ExternalInput")
    w_q = nc.dram_tensor("w_q", [IN_DIM, QW], BF16, kind="ExternalInput")
    kh_idx = nc.dram_tensor("kh_idx", [NCHUNK, ncalls, TILE, CALL // 16], I16,
                            kind="ExternalInput")
    qx_idx = nc.dram_tensor("qx_idx", [NCHUNK, ncalls, TILE, CALL // 16], I16,
                            kind="ExternalInput")
    s_tiles = nc.dram_tensor("s_tiles", [NCHUNK, ncalls, TILE, CALL], F16,
                             kind="ExternalInput")
    res = nc.dram_tensor("res", [B * TILE, OUT_DIM], BF16, kind="ExternalOutput")
    KHW2 = 2 * QW
    KHt = [nc.dram_tensor(f"KH{k}", [NKPAD, KHW2], F16, kind="Internal")
           for k in range(NCHUNK)]
    QT = nc.dram_tensor("QT", [NLOC, QW], Q_DT, kind="Internal")

    with tile.TileContext(nc) as tc:
        with (
            tc.tile_pool(name="dense", bufs=3) as dp,
            tc.tile_pool(name="psum", bufs=4, space="PSUM") as pp,
        ):
            # ---------------- dense phase ----------------
            wkh_t = dp.tile([IN_DIM, KHW], BF16, tag="wkh")
            nc.sync.dma_start(out=wkh_t[:], in_=w_kh[:, :])
            wq_t = dp.tile([IN_DIM, QW], BF16, tag="wq")
            nc.sync.dma_start(out=wq_t[:], in_=w_q[:, :])

            XB = 8
            nt = NKPAD // TILE
            for k in range(NCHUNK):
                for j0 in range(0, nt, XB):
                    jn = min(XB, nt - j0)
                    xt = dp.tile([IN_DIM, XB * TILE], BF16, tag="xt")
                    nc.sync.dma_start(out=xt[:, :jn * TILE],
                                      in_=xTc[k, :, j0 * TILE:(j0 + jn) * TILE])
                    st_k = dp.tile([TILE, XB, QW], K_DT, tag="stk")
                    st_h = dp.tile([TILE, XB, QW], H_DT, tag="sth")
                    for j in range(jn):
                        ps = pp.tile([TILE, KHW], F32, tag="pdense")
                        nc.tensor.matmul(ps[:], lhsT=xt[:, j * TILE:(j + 1) * TILE],
                                         rhs=wkh_t[:], start=True, stop=True)
                        _prelu(nc, dp, st_k[:, j, :], ps[:, 0:QW], f"d{k}_{j0}_{j}")
                        nc.vector.tensor_copy(st_h[:, j, :], ps[:, QW:KHW])
                    nc.sync.dma_start(
                        out=KHt[k][j0 * TILE:(j0 + jn) * TILE, 0:QW]
                        .rearrange("(j p) d -> p j d", p=TILE),
                        in_=st_k[:, :jn, :])
                    nc.sync.dma_start(
                        out=KHt[k][j0 * TILE:(j0 + jn) * TILE, QW:KHW2]
                        .rearrange("(j p) d -> p j d", p=TILE),
                        in_=st_h[:, :jn, :])
            ntl = NLOC // TILE
            for j0 in range(0, ntl, XB):
                jn = min(XB, ntl - j0)
                xt = dp.tile([IN_DIM, XB * TILE], BF16, tag="xt")
                nc.sync.dma_start(out=xt[:, :jn * TILE],
                                  in_=xTloc[:, j0 * TILE:(j0 + jn) * TILE])
                st = dp.tile([TILE, XB, QW], Q_DT, tag="qst")
                for j in range(jn):
                    ps = pp.tile([TILE, QW], F32, tag="pq")
                    nc.tensor.matmul(ps[:], lhsT=xt[:, j * TILE:(j + 1) * TILE],
                                     rhs=wq_t[:], start=True, stop=True)
                    _prelu(nc, dp, st[:, j, :], ps[:], f"q{j0}_{j}")
                nc.sync.dma_start(
                    out=QT[j0 * TILE:(j0 + jn) * TILE, :].rearrange(
                        "(j p) d -> p j d", p=TILE),
                    in_=st[:, :jn, :])

        with (
            tc.tile_pool(name="gth", bufs=3) as gp,
            tc.tile_pool(name="cmp", bufs=2) as cp,
            tc.tile_pool(name="res", bufs=2) as rp,
            tc.tile_pool(name="aux", bufs=2) as ap,
            tc.tile_pool(name="spsum", bufs=4, space="PSUM") as pp2,
            tc.tile_pool(name="outp", bufs=2) as op,
        ):
            # ---------------- edge phase ----------------
            state = {}

            def issue_call(k, j):
                it1t = ap.tile([TILE, CALL // 16], I16, tag="khidx")
                nc.sync.dma_start(out=it1t[:], in_=kh_idx[k, j, :, :])
                it2t = ap.tile([TILE, CALL // 16], I16, tag="qxidx")
                nc.sync.dma_start(out=it2t[:], in_=qx_idx[k, j, :, :])
                st_t = rp.tile([TILE, CPB, TILE], F16, tag=f"sfull{k}")
                nc.sync.dma_start(
                    out=st_t[:],
                    in_=s_tiles[k, j, :, :].rearrange("p (t d) -> p t d", d=TILE))
                khg = gp.tile([TILE, CPB, KHW2], F16, tag="khg")
                nc.gpsimd.dma_gather(
                    out_ap=khg[:], in_ap=KHt[k][:, :], idxs_ap=it1t[:],
                    num_idxs=CALL, num_idxs_reg=CALL, elem_size=KHW2,
                    single_packet=False, queue_num=(2 * j) % nq)
                qxg = gp.tile([TILE, CPB, QW], Q_DT, tag="qxg")
                nc.gpsimd.dma_gather(
                    out_ap=qxg[:], in_ap=QT[:, :], idxs_ap=it2t[:],
                    num_idxs=CALL, num_idxs_reg=CALL, elem_size=QW,
                    single_packet=False, queue_num=(2 * j + 1) % nq)
                kv = khg[:, :, 0:QW]                     # [128, CPB, 256]
                hv = khg[:, :, QW:KHW2]                  # [128, CPB, 256]
                # scores: product in f16, tree reduce
                qk = cp.tile([TILE, CPB, QW], F16, tag="qk")
                nc.vector.tensor_tensor(out=qk[:], in0=qxg[:], in1=kv,
                                        op=mybir.AluOpType.mult)
                cur = qk[:].rearrange("p c (h d) -> p (c h) d", d=OUT_DIM)
                w = OUT_DIM
                with nc.allow_low_precision(reason="f16 tree reduce"):
                    while w > 2:
                        nxt = cp.tile([TILE, CPB * N_HEADS, w // 2], F16,
                                      tag=f"tr{w}")
                        nc.vector.tensor_tensor(out=nxt[:], in0=cur[:, :, 0:w // 2],
                                                in1=cur[:, :, w // 2:w],
                                                op=mybir.AluOpType.add)
                        cur = nxt[:]
                        w //= 2
                s4 = cp.tile([TILE, CPB * N_HEADS], F16, tag="s4")
                with nc.allow_low_precision(reason="f16 score"):
                    nc.vector.tensor_tensor(
                        out=s4[:], in0=cur[:, :, 0], in1=cur[:, :, 1],
                        op=mybir.AluOpType.add)
                # rhs: [esH | es] in one contiguous tile (APs stay <=4D)
                rhs = rp.tile([TILE, CPB, RHSW], F16, tag=f"rhs{k}")
                nc.scalar.activation(
                    rhs[:, :, QW:RHSW],
                    s4[:].rearrange("p (c h) -> p c h", h=N_HEADS),
                    mybir.ActivationFunctionType.Exp,
                    scale=1.0 / (OUT_DIM ** 0.5))
                nc.vector.tensor_tensor(
                    out=rhs[:, :, 0:QW].rearrange("p c (h d) -> p c h d",
                                                  d=OUT_DIM),
                    in0=hv.rearrange("p c (h d) -> p c h d", d=OUT_DIM),
                    in1=rhs[:, :, QW:RHSW].to_broadcast(
                        [TILE, CPB, N_HEADS, OUT_DIM]),
                    op=mybir.AluOpType.mult)
                return st_t, rhs

            def get_call(k, j):
                if (k, j) not in state:
                    state[(k, j)] = issue_call(k, j)
                return state[(k, j)]

            ostage = None
            OB = 8
            for b in range(B):
                if b % OB == 0:
                    ostage = op.tile([TILE, OB, OUT_DIM], BF16, tag="ostage")
                psb = pp2.tile([TILE, RHSW], F32, tag="pblk")
                for k in range(NCHUNK):
                    for t in range(tpc):
                        gslot = (b * tpc + t) * TILE
                        j, colr = divmod(gslot, CALL)
                        ct = colr // TILE
                        st_t, rhs = get_call(k, j)
                        nc.tensor.matmul(psb[:], lhsT=st_t[:, ct, :],
                                         rhs=rhs[:, ct, :],
                                         start=(k == 0 and t == 0),
                                         stop=(k == NCHUNK - 1 and t == tpc - 1))
                # drain
                zt = ap.tile([TILE, N_HEADS], F32, tag="zt")
                nc.vector.tensor_scalar(out=zt[:], in0=psb[:, QW:RHSW],
                                        scalar1=float(N_HEADS), scalar2=1e-19,
                                        op0=mybir.AluOpType.mult,
                                        op1=mybir.AluOpType.add)
                rz = ap.tile([TILE, N_HEADS, 1], F32, tag="rz")
                nc.vector.reciprocal(rz[:, :, 0], zt[:])
                sb = ap.tile([TILE, QW], BF16, tag="sbdrain")
                nc.scalar.activation(sb[:], psb[:, 0:QW],
                                     mybir.ActivationFunctionType.Copy)
                t4 = ap.tile([TILE, N_HEADS, OUT_DIM], BF16, tag="t4")
                with nc.allow_low_precision(reason="head mean"):
                    nc.vector.tensor_tensor(
                        out=t4[:],
                        in0=sb[:].rearrange("p (h d) -> p h d", d=OUT_DIM),
                        in1=rz[:].to_broadcast([TILE, N_HEADS, OUT_DIM]),
                        op=mybir.AluOpType.mult)
                    t2 = ap.tile([TILE, 2, OUT_DIM], BF16, tag="t2")
                    nc.vector.tensor_tensor(out=t2[:], in0=t4[:, 0:2, :],
                                            in1=t4[:, 2:4, :],
                                            op=mybir.AluOpType.add)
                    nc.vector.tensor_tensor(out=ostage[:, b % OB, :],
                                            in0=t2[:, 0, :], in1=t2[:, 1, :],
                                            op=mybir.AluOpType.add)
                if b % OB == OB - 1 or b == B - 1:
                    b0 = (b // OB) * OB
                    bn = b - b0 + 1
                    nc.sync.dma_start(
                        out=res[b0 * TILE:(b0 + bn) * TILE, :].rearrange(
                            "(j p) d -> p j d", p=TILE),
                        in_=ostage[:, :bn, :])
    nc.compile()
    return nc


# ------------------------------------------------------------ entry point

_CACHE = {}


def _pick_tpc(edge_index, n_nodes, base=3):
    row = np.asarray(edge_index[0], dtype=np.int64)
    col = np.asarray(edge_index[1], dtype=np.int64)
    key = row * NCHUNK + (col % NCHUNK)
    mx = int(np.bincount(key, minlength=n_nodes * NCHUNK).max())
    return max(base, -(-mx // TILE))


def kernel(x, edge_index, Wq, Wk, Wh, bh):
    x = np.asarray(x)
    edge_index = np.asarray(edge_index)
    meta, per_core = preprocess(x, edge_index, Wq, Wk, Wh, bh,
                                tiles_per_chunk=_pick_tpc(edge_index, x.shape[0]))
    key = (meta["NKPAD"], meta["NLOC"], meta["B"], meta["SCP"],
           meta["tiles_per_chunk"])
    if key not in _CACHE:
        nc = build_kernel(meta)
        _CACHE[key] = AxRunner(nc, N_CORES)
    r = _CACHE[key]
    r.put_inputs(per_core)
    results = r.results(r.run())
    return assemble(meta, results, bh).astype(np.float32)
